# revision 11
# baseline (speedup 1.0000x reference)
"""Trainium2 Bass kernel for nn_AbstractorCore (6-layer abstractor transformer).

Sharding: 8 cores = 4 batches x 2 token-halves. Core c owns 512 tokens of
batch c//2. Cross-attention q,k derive from the static g (local); v and
self-attention k,v need the full batch's evolving x, exchanged pairwise via
AllReduce(add) of the RAW bf16 x (2 per layer). The partner half is
recovered position-independently as other = sum - own (~1 ulp fp noise),
and its LN stats are recomputed locally so every other-half GEMM can run
stats-folded on the raw tensor (no normalize on the exchange critical path;
the AllReduce issues as soon as the preceding FF's residual lands).

Layouts: activations are feature-major (FM): X^T as [128, F//128, T] SBUF
tiles. GEMMs use natural-layout weights as lhsT (FM output) or the FM
activation as lhsT (token-major output, used for V). LN scale/bias are folded
into adjacent weights host-side, so on-chip LN is a pure normalize whose
per-token stats come from ones-matrix matmuls (stats replicated across all
partitions -> no partition broadcasts). Softmax skips max-subtraction
(|dots*scale| = O(1) here); the denominator comes from a ones-column appended
to V in the P@V matmul; 1/den = exp(-ln(den)) on ScalarE.

Matmul operands are bf16 (fp32 matmul lowers to 3-pass fp32r on TRN2);
accumulation and the residual stream x stay fp32.
"""
import numpy as np

import concourse.bass as bass
import concourse.mybir as mybir
import concourse.tile as tile
from concourse import bacc
from concourse import bass_utils

AF = mybir.ActivationFunctionType
OP = mybir.AluOpType
FP = mybir.dt.float32
BF = mybir.dt.bfloat16

B, N, DIM, HEADS, DHEAD, MLP, DEPTH = 4, 1024, 512, 8, 64, 2048, 6
INNER = HEADS * DHEAD
SCALE = DHEAD ** -0.5
EPS = 1e-5
P = 128
KC = DIM // P          # 4 contraction chunks of 128
TOWN = N // 2          # 512 tokens per core
NT = N // P            # 8 k-token chunks per full batch
HP = HEADS // 2        # 4 head pairs
MC = MLP // P          # 16
N_CORES = 8
RG = [[0, 1], [2, 3], [4, 5], [6, 7]]


def _rearr(dram_2d):
    """[D, F] dram AP -> [128, D//128, F] (contraction chunks on partitions)."""
    return dram_2d.rearrange("(ko ki) f -> ki ko f", ki=P)


def _bcast(row, parts):
    """[1, T] DRAM AP -> [parts, T] partition-broadcast AP (DMA source only)."""
    return bass.AP(tensor=row.tensor, offset=row.offset,
                   ap=[[0, parts]] + list(row.ap[1:]))


def _patch_act_tables():
    """Strip Exp/Ln from the earlier table sets so both resolve to the shared
    natural_log_exp_and_others set -> no ACT table reload between Ln and Exp
    (set ids keep their positions, only membership changes)."""
    from concourse import hw_specs
    import concourse.bacc as bacc_mod
    if getattr(bacc_mod, "_act_tables_patched", False):
        return
    orig = hw_specs.get_activation_tables

    def patched(arch):
        t = {}
        for k, v in orig(arch).items():
            if k in ("exp_and_others", "natural_log"):
                v = v - {AF.Exp, AF.Ln}
            t[k] = v
        return t

    bacc_mod.get_activation_tables = patched
    bacc_mod._act_tables_patched = True


def build(depth=DEPTH):
    _patch_act_tables()
    nc = bacc.Bacc("TRN2", target_bir_lowering=False, debug=False,
                   enable_asserts=False, num_devices=N_CORES)

    g_d = nc.dram_tensor("g_fm", [DIM, N], FP, kind="ExternalInput").ap()
    x_d = nc.dram_tensor("x_fm", [DIM, TOWN], FP, kind="ExternalInput").ap()
    wdr = {}
    for nm, rows, cols in [("Wq", DIM, INNER), ("Wk", DIM, INNER),
                           ("Wv", DIM, INNER), ("Wo_ca", INNER, DIM),
                           ("Wq_sa", DIM, INNER), ("Wk_sa", DIM, INNER),
                           ("Wv_sa", DIM, INNER), ("Wo_sa", INNER, DIM),
                           ("W1", DIM, MLP), ("W2", MLP, DIM)]:
        wdr[nm] = nc.dram_tensor(nm, [depth, rows, cols], BF,
                                 kind="ExternalInput").ap()
    w1cs_d = nc.dram_tensor("W1cs", [depth, 1, MLP], BF,
                            kind="ExternalInput").ap()
    wcs_d = nc.dram_tensor("Wcs", [depth, 4, INNER], BF,
                           kind="ExternalInput").ap()
    out_d = nc.dram_tensor("x_out", [DIM, TOWN], FP, kind="ExternalOutput").ap()

    with tile.TileContext(nc) as tc:
        with (
            tc.tile_pool(name="persist", bufs=1) as persist,
            tc.tile_pool(name="big", bufs=1) as bigp,
            tc.tile_pool(name="act", bufs=1) as act,
            tc.tile_pool(name="actn", bufs=2) as actn,
            tc.tile_pool(name="wnext", bufs=2) as wnp,
            tc.tile_pool(name="zn", bufs=2) as znp,
            tc.tile_pool(name="oth", bufs=1) as othp,
            tc.tile_pool(name="pt", bufs=8) as ptp,
            tc.tile_pool(name="sq", bufs=1) as sqp,
            tc.tile_pool(name="w", bufs=4) as wp,
            tc.tile_pool(name="wff", bufs=1) as wffp,
            tc.tile_pool(name="srow", bufs=4) as srow,
            tc.tile_pool(name="den", bufs=2) as denp,
            tc.tile_pool(name="nst", bufs=4) as nstp,
            tc.tile_pool(name="brow", bufs=2) as browp,
            tc.tile_pool(name="ps_mm", bufs=2, space="PSUM") as ps_mm,
            tc.tile_pool(name="ps_ss", bufs=4, space="PSUM") as ps_ss,
            tc.tile_pool(name="ps_pv", bufs=2, space="PSUM") as ps_pv,
            tc.tile_pool(name="dram", bufs=2, space="DRAM") as dramp,
        ):
            ones = persist.tile([P, P], FP)   # ones matrix: replicated stats
            nc.vector.memset(ones[:], 1.0)
            ones_bf = persist.tile([P, P], BF)
            nc.vector.memset(ones_bf[:], 1.0)
            eps_col = persist.tile([P, 1], FP)
            nc.vector.memset(eps_col[:], EPS)
            dummy_act = persist.tile([1, 1], FP)
            nc.vector.memset(dummy_act[:], 1.0)
            x_own = persist.tile([P, KC, TOWN], FP)          # x^T own half
            nc.sync.dma_start(x_own[:], _rearr(x_d))
            xb_own = persist.tile([P, KC, TOWN], BF)         # bf16 shadow
            for kc in range(KC):
                nc.scalar.activation(xb_own[:, kc, :], x_own[:, kc, :],
                                     AF.Copy)
            gn = persist.tile([P, KC, N], BF)                # normalize(g)^T

            def hoist_table(func):
                """Tiny dummy activation: forces the ACT table load for
                `func`'s set to happen HERE in the ScalarE queue instead of
                at the next real use (hoists it off the critical chain)."""
                nc.scalar.activation(dummy_act[:], dummy_act[:], func)

            def stats_fm(stat_src, stat_ones, fp32_stats=False):
                """Per-token LN stats of a [P, KC, 512] FM tile.

                Returns (a, cb, aT): a = rstd row (replicated, fp32), cb =
                bf16 -mean*rstd row (K=1 matmul operand), aT = token-major
                rstd columns (via a tiny DRAM transpose bounce)."""
                T = stat_src.shape[2]
                sq = sqp.tile([P, KC, T], stat_src.dtype, tag="sq")
                for kc in range(KC):
                    nc.vector.tensor_tensor(sq[:, kc, :], stat_src[:, kc, :],
                                            stat_src[:, kc, :], OP.mult)
                s_ps = ps_mm.tile([P, T], FP, tag="mm")
                q_ps = ps_mm.tile([P, T], FP, tag="mm")
                for kc in range(KC):
                    nc.tensor.matmul(s_ps[:], stat_ones[:],
                                     stat_src[:, kc, :],
                                     start=kc == 0, stop=kc == KC - 1)
                for kc in range(KC):
                    nc.tensor.matmul(q_ps[:], stat_ones[:], sq[:, kc, :],
                                     start=kc == 0, stop=kc == KC - 1)
                nm = srow.tile([P, T], FP, tag="srow")
                ms = srow.tile([P, T], FP, tag="srow")
                nc.vector.tensor_scalar_mul(nm[:], s_ps[:], -1.0 / DIM)
                nc.vector.tensor_scalar_mul(ms[:], q_ps[:], 1.0 / DIM)
                var = srow.tile([P, T], FP, tag="srow")
                nc.vector.tensor_tensor(var[:], nm[:], nm[:], OP.mult)
                nc.vector.tensor_tensor(var[:], ms[:], var[:], OP.subtract)
                a = nstp.tile([P, T], FP, tag="sta")
                c = nstp.tile([P, T], FP, tag="stc", bufs=2)
                nc.scalar.activation(a[:], var[:], AF.Ln, bias=eps_col[:])
                nc.scalar.activation(a[:], a[:], AF.Exp, scale=-0.5)
                nc.vector.tensor_tensor(c[:], nm[:], a[:], OP.mult)
                cb = nstp.tile([1, T], BF, tag="stcb")
                nc.vector.tensor_copy(out=cb[:], in_=c[0:1, :])
                ar = dramp.tile([1, T], FP, tag="arow")
                nc.sync.dma_start(ar[:], a[0:1, :])
                aT = nstp.tile([P, T // P], FP, tag="staT")
                nc.sync.dma_start(
                    aT[:], ar[0, :].rearrange("(o p) -> p o", p=P))
                return (a, c, cb, aT)

            def norm_apply(src, a, c, dst):
                """dst(bf16) = src*a + c, FM tiles [P, KC, 512]."""
                tmp = srow.tile([P, 512], FP, tag="srow")
                for kc in range(KC):
                    nc.vector.tensor_tensor(tmp[:], src[:, kc, :],
                                            a[:], OP.mult)
                    nc.vector.tensor_tensor(dst[:, kc, :], tmp[:], c[:],
                                            OP.add)

            def norm_fm_g(src, dst):
                """dst(bf16) = normalize(src fp32) for the static g (T=N)."""
                for c0 in range(0, N, 512):
                    sq = sqp.tile([P, KC, 512], BF, tag="sq")
                    for kc in range(KC):
                        nc.vector.tensor_tensor(sq[:, kc, :],
                                                src[:, kc, c0:c0 + 512],
                                                src[:, kc, c0:c0 + 512],
                                                OP.mult)
                    s_ps = ps_mm.tile([P, 512], FP, tag="mm")
                    q_ps = ps_mm.tile([P, 512], FP, tag="mm")
                    for kc in range(KC):
                        nc.tensor.matmul(s_ps[:], ones[:],
                                         src[:, kc, c0:c0 + 512],
                                         start=kc == 0, stop=kc == KC - 1)
                    for kc in range(KC):
                        nc.tensor.matmul(q_ps[:], ones_bf[:], sq[:, kc, :],
                                         start=kc == 0, stop=kc == KC - 1)
                    nm = srow.tile([P, 512], FP, tag="srow")
                    ms = srow.tile([P, 512], FP, tag="srow")
                    nc.vector.tensor_scalar_mul(nm[:], s_ps[:], -1.0 / DIM)
                    nc.vector.tensor_scalar_mul(ms[:], q_ps[:], 1.0 / DIM)
                    var = srow.tile([P, 512], FP, tag="srow")
                    nc.vector.tensor_tensor(var[:], nm[:], nm[:], OP.mult)
                    nc.vector.tensor_tensor(var[:], ms[:], var[:], OP.subtract)
                    a = srow.tile([P, 512], FP, tag="srow")
                    c = srow.tile([P, 512], FP, tag="srow")
                    nc.scalar.activation(a[:], var[:], AF.Ln, bias=eps_col[:])
                    nc.scalar.activation(a[:], a[:], AF.Exp, scale=-0.5)
                    nc.vector.tensor_tensor(c[:], nm[:], a[:], OP.mult)
                    tmp = srow.tile([P, 512], FP, tag="srow")
                    for kc in range(KC):
                        nc.vector.tensor_tensor(tmp[:],
                                                src[:, kc, c0:c0 + 512],
                                                a[:], OP.mult)
                        nc.vector.tensor_tensor(dst[:, kc, c0:c0 + 512],
                                                tmp[:], c[:], OP.add)

            def load_w(dram_slice):
                t = wp.tile([P, KC, 512], BF, tag="w")
                nc.sync.dma_start(t[:], _rearr(dram_slice))
                return t

            def gemm_fm(w_tile, src, dst, T, t_dst0=0, t_src0=0, evac="act"):
                """dst[:, ft, t_dst0+t] = (W^T @ src), FM output, bf16.

                evac picks the PSUM->SBUF engine: "act" for GEMM/norm phases
                (ScalarE idle there), "dve" near attention (ScalarE is busy
                with softmax exp and would stall the PV pipeline)."""
                Fts = w_tile.shape[2] // P
                for ft in range(Fts):
                    for t0 in range(0, T, 512):
                        ps = ps_mm.tile([P, 512], FP, tag="mm")
                        for kc in range(KC):
                            nc.tensor.matmul(
                                ps[:], w_tile[:, kc, ft * P:(ft + 1) * P],
                                src[:, kc, t_src0 + t0:t_src0 + t0 + 512],
                                start=kc == 0, stop=kc == KC - 1)
                        dsl = dst[:, ft, t_dst0 + t0:t_dst0 + t0 + 512]
                        if evac == "act":
                            nc.scalar.activation(dsl, ps[:], AF.Copy)
                        else:
                            nc.vector.tensor_copy(out=dsl, in_=ps[:])

            def gemm_fm_folded(w_tile, src_bf, dst, wcs, wi, stats,
                               t_dst0=0, evac="dve"):
                """FM GEMM on the RAW bf16 x-shadow; the per-token normalize
                (a, c) is folded in: c via a K=1 matmul against the weight
                column sums, a via the evacuation multiply. Removes the LN
                apply from the GEMM critical path entirely."""
                a, c, cb, aT = stats
                Fts = w_tile.shape[2] // P
                for ft in range(Fts):
                    ps = ps_mm.tile([P, 512], FP, tag="mm")
                    for kc in range(KC):
                        nc.tensor.matmul(ps[:], w_tile[:, kc, ft * P:(ft + 1) * P],
                                         src_bf[:, kc, :],
                                         start=kc == 0, stop=False)
                    nc.tensor.matmul(ps[:], wcs[0:1, wi, ft * P:(ft + 1) * P],
                                     cb[:], start=False, stop=True)
                    dsl = dst[:, ft, t_dst0:t_dst0 + 512]
                    nc.vector.tensor_tensor(dsl, ps[:], a[:], OP.mult)

            def gemm_vcat_folded(w_tile, src_bf, vcat, wcs, wi, stats, tt0=0):
                a, c, cb, aT = stats
                for tt in range(KC):
                    ps = ps_mm.tile([P, 512], FP, tag="mm")
                    for kc in range(KC):
                        nc.tensor.matmul(ps[:], src_bf[:, kc, tt * P:(tt + 1) * P],
                                         w_tile[:, kc, :],
                                         start=kc == 0, stop=False)
                    nc.tensor.matmul(ps[:], cb[0:1, tt * P:(tt + 1) * P],
                                     wcs[0:1, wi, :], start=False, stop=True)
                    nc.vector.tensor_scalar_mul(
                        vcat[:, tt0 + tt, :, 0:DHEAD],
                        ps.rearrange("p (h d) -> p h d", h=HEADS),
                        aT[:, tt:tt + 1])

            def attention(qT, kT, vcat, merged, mid_cb=None, mid_kc=KC):
                """merged (FM bf16 [128, KC, 512]) = softmax(qk^T*scale)@v.

                mid_cb is invoked after `mid_kc` own-half k-chunks of the
                first head pair: the emitted instructions (other-half k/v
                GEMMs, which wait on the AllReduce) land behind own-half PE
                work in the static per-engine order, hiding the exchange."""
                for hp in range(HP):
                    pv0 = ps_pv.tile([DHEAD + 1, 512], FP, tag="pv")
                    pv1 = ps_pv.tile([DHEAD + 1, 512], FP, tag="pv")
                    pts = [None] * NT

                    def emit_pv(kc, pv0=pv0, pv1=pv1, pts=pts, vcat=vcat):
                        nc.tensor.matmul(pv0[:], vcat[:, kc, 2 * hp, :],
                                         pts[kc][0][:],
                                         start=kc == 0, stop=kc == NT - 1)
                        nc.tensor.matmul(pv1[:], vcat[:, kc, 2 * hp + 1, :],
                                         pts[kc][1][:],
                                         start=kc == 0, stop=kc == NT - 1)

                    for kc in range(NT):
                        if mid_cb is not None and hp == 0 and kc == mid_kc:
                            mid_cb()
                            mid_cb = None
                        ss0 = ps_ss.tile([P, 512], FP, tag="ss")
                        ss1 = ps_ss.tile([P, 512], FP, tag="ss")
                        nc.tensor.matmul(ss0[:],
                                         kT[0:DHEAD, hp, kc * P:(kc + 1) * P],
                                         qT[0:DHEAD, hp, :],
                                         start=True, stop=True)
                        nc.tensor.matmul(ss1[:],
                                         kT[DHEAD:P, hp, kc * P:(kc + 1) * P],
                                         qT[DHEAD:P, hp, :],
                                         start=True, stop=True)
                        pt0 = ptp.tile([P, 512], BF, tag="pt")
                        pt1 = ptp.tile([P, 512], BF, tag="pt")
                        nc.scalar.activation(pt0[:], ss0[:], AF.Exp,
                                             scale=SCALE)
                        nc.scalar.activation(pt1[:], ss1[:], AF.Exp,
                                             scale=SCALE)
                        pts[kc] = (pt0, pt1)
                        # lag PV two steps behind so it never head-of-line
                        # blocks on its own exp
                        if kc >= 2:
                            emit_pv(kc - 2)
                    emit_pv(NT - 2)
                    emit_pv(NT - 1)
                    # evacuate PV unnormalized immediately (releases the pv
                    # psum slots for the next pair); normalize in place after
                    den_sb = denp.tile([1, 1024], FP, tag="densb")
                    nc.vector.tensor_copy(out=den_sb[:, 0:512],
                                          in_=pv0[DHEAD:DHEAD + 1, :])
                    nc.vector.tensor_copy(out=den_sb[:, 512:1024],
                                          in_=pv1[DHEAD:DHEAD + 1, :])
                    nc.vector.tensor_copy(out=merged[0:DHEAD, hp, :],
                                          in_=pv0[0:DHEAD, :])
                    nc.vector.tensor_copy(out=merged[DHEAD:P, hp, :],
                                          in_=pv1[0:DHEAD, :])
                    r01 = denp.tile([1, 1024], FP, tag="den")
                    nc.vector.reciprocal_approx_fast(out=r01[:], in_=den_sb[:])
                    rd = dramp.tile([2, 512], FP, tag="rrow")
                    nc.sync.dma_start(rd[:].rearrange("a b -> (a b)")[None, :],
                                      r01[:])
                    rb = browp.tile([P, 512], FP, tag="brow")
                    nc.sync.dma_start(rb[0:DHEAD, :], _bcast(rd[0:1, :], DHEAD))
                    nc.sync.dma_start(rb[DHEAD:P, :], _bcast(rd[1:2, :], DHEAD))
                    nc.vector.tensor_tensor(merged[0:DHEAD, hp, :],
                                            merged[0:DHEAD, hp, :],
                                            rb[0:DHEAD, :], OP.mult)
                    nc.vector.tensor_tensor(merged[DHEAD:P, hp, :],
                                            merged[DHEAD:P, hp, :],
                                            rb[DHEAD:P, :], OP.mult)

            def gemm_residual(w_tile, src):
                """x_own += src^T @ W  (W [DIM, DIM] natural as lhsT)."""
                for d in range(KC):
                    ps = ps_mm.tile([P, 512], FP, tag="mm")
                    for kc in range(KC):
                        nc.tensor.matmul(ps[:], w_tile[:, kc, d * P:(d + 1) * P],
                                         src[:, kc, :],
                                         start=kc == 0, stop=kc == KC - 1)
                    nc.vector.tensor_tensor(x_own[:, d, :], ps[:],
                                            x_own[:, d, :], OP.add)
                    nc.scalar.activation(xb_own[:, d, :], x_own[:, d, :],
                                         AF.Copy)

            def ff(w1, w2, zf):
                h = bigp.tile([P, MC, TOWN], BF, tag="h")
                # W2 accumulates all 4 output d-tiles in parallel (borrowing
                # the attention ss psum slots, idle during FF) with the
                # k-chunk loop OUTERMOST: each W2 matmul issues as soon as
                # its gelu chunk lands instead of after the whole h tensor.
                accs = [ps_ss.tile([P, 512], FP, tag="ss", name=f"acc{d}")
                        for d in range(KC)]
                for ft in range(MC):
                    ps = ps_mm.tile([P, 512], FP, tag="mm")
                    for kc in range(KC):
                        nc.tensor.matmul(ps[:], w1[:, kc, ft * P:(ft + 1) * P],
                                         zf[:, kc, :],
                                         start=kc == 0, stop=kc == KC - 1)
                    nc.scalar.activation(h[:, ft, :], ps[:], AF.Gelu)
                    for d in range(KC):
                        nc.tensor.matmul(accs[d][:],
                                         w2[:, ft, d * P:(d + 1) * P],
                                         h[:, ft, :],
                                         start=ft == 0, stop=ft == MC - 1)
                # hoist the ln/exp table reload behind the W2 tail: the next
                # phase's first Ln would otherwise pay it on the stats chain
                hoist_table(AF.Ln)
                for d in range(KC):
                    nc.vector.tensor_tensor(x_own[:, d, :], accs[d][:],
                                            x_own[:, d, :], OP.add)
                    nc.scalar.activation(xb_own[:, d, :], x_own[:, d, :],
                                         AF.Copy)

            def exchange():
                """Pairwise AllReduce(add) of the raw bf16 x shadow."""
                bi = dramp.tile([DIM, TOWN], BF, tag="agin")
                bo = dramp.tile([DIM, TOWN], BF, tag="agout")
                nc.sync.dma_start(_rearr(bi[:]), xb_own[:])
                nc.gpsimd.collective_compute(
                    "AllReduce", OP.add, ins=[bi.opt()], outs=[bo.opt()],
                    replica_groups=RG)
                return bo

            def assemble_other(bo):
                """other = pairsum - own   (position-independent)."""
                zo = othp.tile([P, KC, TOWN], BF, tag="znoth")
                nc.sync.dma_start(zo[:], _rearr(bo[0:DIM, :]))
                for kc in range(KC):
                    nc.vector.tensor_tensor(zo[:, kc, :], zo[:, kc, :],
                                            xb_own[:, kc, :], OP.subtract)
                return zo

            def ham_warm(n=16):
                """Dummy bf16 matmuls to keep the PE HAM clock-gate hot
                across norm gaps where no real PE work is available."""
                ps = ps_ss.tile([P, 512], FP, tag="ss")
                for _ in range(n):
                    nc.tensor.matmul(ps[:], ones_bf[:], gn[:, 0, 0:512],
                                     start=True, stop=True)

            # ---- prologue: first exchange + static gn = normalize(g) ----
            bo_ca = exchange()
            st_end = stats_fm(xb_own, ones_bf)
            g_fm = bigp.tile([P, KC, N], FP, tag="h")  # reuse h slot
            nc.sync.dma_start(g_fm[:], _rearr(g_d))
            norm_fm_g(g_fm, gn)

            for i in range(depth):
                # ======== relational cross attention ========
                if i == 0:
                    wq = load_w(wdr["Wq"][0])
                    wk = load_w(wdr["Wk"][0])
                    qT = actn.tile([P, KC, 512], BF, tag="qT")
                    kT = actn.tile([P, KC, N], BF, tag="kT")
                    gemm_fm(wq, gn, qT, 512)          # own queries (local)
                    gemm_fm(wk, gn, kT, N)            # all keys (g static)
                else:
                    qT, kT = qT_next, kT_next
                # all of this layer's weight DMAs up front, in consumption
                # order: they roll through the pools as prefetch so no GEMM
                # waits on a just-issued transfer.
                wv = load_w(wdr["Wv"][i])
                if i + 1 < depth:  # next-layer fill weights
                    wk_n = wnp.tile([P, KC, 512], BF, tag="wn")
                    nc.sync.dma_start(wk_n[:], _rearr(wdr["Wk"][i + 1]))
                    wq_n = wnp.tile([P, KC, 512], BF, tag="wn")
                    nc.sync.dma_start(wq_n[:], _rearr(wdr["Wq"][i + 1]))
                woc = load_w(wdr["Wo_ca"][i])
                w1 = wffp.tile([P, KC, MLP], BF, tag="w1")
                nc.sync.dma_start(w1[:], _rearr(wdr["W1"][i]))
                w2 = wffp.tile([P, MC, DIM], BF, tag="w2")
                nc.sync.dma_start(w2[:], _rearr(wdr["W2"][i]))
                w1cs = othp.tile([1, MLP], BF, tag="w1cs")
                nc.sync.dma_start(w1cs[:], w1cs_d[i])
                wcs = othp.tile([1, 4, INNER], BF, tag="wcs")
                nc.sync.dma_start(wcs[:], wcs_d[i])
                wqs = load_w(wdr["Wq_sa"][i])
                wks = load_w(wdr["Wk_sa"][i])
                wvs = load_w(wdr["Wv_sa"][i])
                wos = load_w(wdr["Wo_sa"][i])
                vcat = act.tile([P, NT, HEADS, DHEAD + 1], BF, tag="vcat")
                nc.vector.memset(vcat[:, :, :, DHEAD:DHEAD + 1], 1.0)
                gemm_vcat_folded(wv, xb_own, vcat, wcs, 0, st_end)
                merged = act.tile([P, KC, 512], BF, tag="merged")

                def ca_mid(bo=bo_ca, wv=wv, wcs=wcs, vc=vcat):
                    zo = assemble_other(bo)
                    st_o = stats_fm(zo, ones_bf)
                    gemm_vcat_folded(wv, zo, vc, wcs, 0, st_o, tt0=KC)
                attention(qT, kT, vcat, merged, mid_cb=ca_mid, mid_kc=KC)
                gemm_residual(woc, merged)
                # ======== feed-forward 1 ========
                # fill the xb-cast + stats gap with next layer's first k half
                ham_warm(4)
                if i + 1 < depth:
                    kT_next = actn.tile([P, KC, N], BF, tag="kT")
                    gemm_fm(wk_n, gn, kT_next, 512)
                st_f1 = stats_fm(xb_own, ones_bf)
                zf = znp.tile([P, KC, TOWN], BF, tag="znown")
                norm_apply(x_own, st_f1[0], st_f1[1], zf)
                hoist_table(AF.Gelu)
                ham_warm(8)
                ff(w1, w2, zf)
                # ======== self attention ========
                bo_sa = exchange()
                ham_warm(4)
                # fill: next layer's queries (gn is static)
                if i + 1 < depth:
                    qT_next = actn.tile([P, KC, 512], BF, tag="qT")
                    gemm_fm(wq_n, gn, qT_next, 512)
                st1 = stats_fm(xb_own, ones_bf)
                ham_warm(6)
                qTs = actn.tile([P, KC, 512], BF, tag="qT")
                kTs = actn.tile([P, KC, N], BF, tag="kT")
                vcats = act.tile([P, NT, HEADS, DHEAD + 1], BF, tag="vcat")
                nc.vector.memset(vcats[:, :, :, DHEAD:DHEAD + 1], 1.0)
                gemm_fm_folded(wqs, xb_own, qTs, wcs, 1, st1)
                gemm_fm_folded(wks, xb_own, kTs, wcs, 2, st1)
                gemm_vcat_folded(wvs, xb_own, vcats, wcs, 3, st1)
                mergeds = act.tile([P, KC, 512], BF, tag="merged")

                def sa_mid(bo=bo_sa, wk_=wks, wv_=wvs, wcs=wcs, kt=kTs,
                           vc=vcats):
                    zo1 = assemble_other(bo)
                    st_o = stats_fm(zo1, ones_bf)
                    gemm_fm_folded(wk_, zo1, kt, wcs, 2, st_o, t_dst0=512)
                    gemm_vcat_folded(wv_, zo1, vc, wcs, 3, st_o, tt0=KC)
                attention(qTs, kTs, vcats, mergeds, mid_cb=sa_mid, mid_kc=KC)
                gemm_residual(wos, mergeds)
                # ======== feed-forward 2 ========
                # fill: next layer's second k half
                ham_warm(4)
                if i + 1 < depth:
                    gemm_fm(wk_n, gn, kT_next, 512, t_dst0=512, t_src0=512)
                st_f2 = stats_fm(xb_own, ones_bf)
                zf2 = znp.tile([P, KC, TOWN], BF, tag="znown")
                norm_apply(x_own, st_f2[0], st_f2[1], zf2)
                hoist_table(AF.Gelu)
                ham_warm(8)
                ff(w1, w2, zf2)
                if i + 1 < depth:
                    bo_ca = exchange()
                    ham_warm(12)
                    st_end = stats_fm(xb_own, ones_bf)
                    ham_warm(8)

            nc.sync.dma_start(_rearr(out_d[:]), x_own[:])

    nc.compile()
    return nc


# ======================= host side =======================

_NC_CACHE = {}


def _get_nc(depth=DEPTH):
    if depth not in _NC_CACHE:
        _NC_CACHE[depth] = build(depth)
    return _NC_CACHE[depth]


def _prep_inputs(inputs, depth=DEPTH):
    import ml_dtypes
    bf16 = ml_dtypes.bfloat16
    f32 = lambda a: np.asarray(a, np.float32)
    g, x = f32(inputs["g"]), f32(inputs["x"])
    lng_s, lnx_s = f32(inputs["lng_s"]), f32(inputs["lnx_s"])
    ln1_s, lnf_s = f32(inputs["ln1_s"]), f32(inputs["lnf_s"])
    # all additive biases must be zero for this kernel (they are, per
    # setup_inputs); LN scales are folded into the adjacent weights.
    for k in ("lng_b", "lnx_b", "ln1_b", "lnf_b", "bv",
              "bo_ca", "bo_sa", "b1", "b2"):
        assert np.abs(f32(inputs[k])).max() == 0.0, f"nonzero bias {k}"
    Wq = lng_s[:, :, None] * f32(inputs["Wq"])
    Wk = lng_s[:, :, None] * f32(inputs["Wk"])
    Wv = lnx_s[:, :, None] * f32(inputs["Wv"])
    Wqkv = ln1_s[:, :, None] * f32(inputs["Wqkv"])
    W1 = lnf_s[:, :, None] * f32(inputs["W1"])
    c = lambda a: np.ascontiguousarray(a.astype(bf16))
    weights = {
        "Wq": c(Wq[:depth]), "Wk": c(Wk[:depth]), "Wv": c(Wv[:depth]),
        "Wo_ca": c(f32(inputs["Wo_ca"])[:depth]),
        "Wq_sa": c(Wqkv[:depth, :, 0:INNER]),
        "Wk_sa": c(Wqkv[:depth, :, INNER:2 * INNER]),
        "Wv_sa": c(Wqkv[:depth, :, 2 * INNER:3 * INNER]),
        "Wo_sa": c(f32(inputs["Wo_sa"])[:depth]),
        "W1": c(W1[:depth]), "W2": c(f32(inputs["W2"])[:depth]),
        "W1cs": c(W1[:depth].astype(bf16).astype(np.float32)
                  .sum(axis=1, keepdims=True)),
    }
    wcs = np.stack([
        weights["Wv"].astype(np.float32).sum(axis=1),
        weights["Wq_sa"].astype(np.float32).sum(axis=1),
        weights["Wk_sa"].astype(np.float32).sum(axis=1),
        weights["Wv_sa"].astype(np.float32).sum(axis=1),
    ], axis=1)
    weights["Wcs"] = c(wcs)
    in_maps = []
    cc = np.ascontiguousarray
    for core in range(N_CORES):
        b, h = core // 2, core % 2
        own = slice(h * TOWN, (h + 1) * TOWN)
        oth = slice((1 - h) * TOWN, (2 - h) * TOWN)
        g_local = np.concatenate([g[b, own], g[b, oth]], axis=0)  # local order
        m = dict(weights)
        m["g_fm"] = cc(g_local.T)
        m["x_fm"] = cc(x[b, own].T)
        in_maps.append(m)
    return in_maps


def _assemble(results):
    out = np.empty((B, N, DIM), np.float32)
    for core in range(N_CORES):
        b, h = core // 2, core % 2
        out[b, h * TOWN:(h + 1) * TOWN] = results[core]["x_out"].T
    return out


def run(inputs, depth=DEPTH, trace=False, tmpdir=None):
    nc = _get_nc(depth)
    in_maps = _prep_inputs(inputs, depth)
    res = bass_utils.run_bass_kernel_spmd(
        nc, in_maps, core_ids=list(range(N_CORES)), trace=trace, tmpdir=tmpdir)
    return _assemble(res.results), res


def kernel(**inputs) -> np.ndarray:
    out, _ = run(inputs)
    return out


# revision 17
# speedup vs baseline: 5.3118x; 5.3118x over previous
"""Trainium2 Bass kernel for nn_AbstractorCore (6-layer abstractor transformer).

Sharding: 8 cores = 4 batches x 2 token-halves. Core c owns 512 tokens of
batch c//2. Cross-attention q,k derive from the static g (local); v and
self-attention k,v need the full batch's evolving x, exchanged pairwise via
AllReduce(add) of the RAW bf16 x (2 per layer). The partner half is
recovered position-independently as other = sum - own (~1 ulp fp noise),
and its LN stats are recomputed locally so every other-half GEMM can run
stats-folded on the raw tensor (no normalize on the exchange critical path;
the AllReduce issues as soon as the preceding FF's residual lands).

Layouts: activations are feature-major (FM): X^T as [128, F//128, T] SBUF
tiles. GEMMs use natural-layout weights as lhsT (FM output) or the FM
activation as lhsT (token-major output, used for V). LN scale/bias are folded
into adjacent weights host-side, so on-chip LN is a pure normalize whose
per-token stats come from ones-matrix matmuls (stats replicated across all
partitions -> no partition broadcasts). Softmax skips max-subtraction
(|dots*scale| = O(1) here); the denominator comes from a ones-column appended
to V in the P@V matmul; 1/den = exp(-ln(den)) on ScalarE.

Matmul operands are bf16 (fp32 matmul lowers to 3-pass fp32r on TRN2);
accumulation and the residual stream x stay fp32.
"""
import numpy as np

import concourse.bass as bass
import concourse.mybir as mybir
import concourse.tile as tile
from concourse import bacc
from concourse import bass_utils

AF = mybir.ActivationFunctionType
OP = mybir.AluOpType
FP = mybir.dt.float32
BF = mybir.dt.bfloat16
F8 = mybir.dt.float8e4
DR = mybir.MatmulPerfMode.DoubleRow

B, N, DIM, HEADS, DHEAD, MLP, DEPTH = 4, 1024, 512, 8, 64, 2048, 6
INNER = HEADS * DHEAD
SCALE = DHEAD ** -0.5
EPS = 1e-5
P = 128
KC = DIM // P          # 4 contraction chunks of 128
TOWN = N // 2          # 512 tokens per core
NT = N // P            # 8 k-token chunks per full batch
HP = HEADS // 2        # 4 head pairs
MC = MLP // P          # 16
N_CORES = 8
RG = [[0, 1], [2, 3], [4, 5], [6, 7]]


def _rearr(dram_2d):
    """[D, F] dram AP -> [128, D//128, F] (contraction chunks on partitions)."""
    return dram_2d.rearrange("(ko ki) f -> ki ko f", ki=P)


def _bcast(row, parts):
    """[1, T] DRAM AP -> [parts, T] partition-broadcast AP (DMA source only)."""
    return bass.AP(tensor=row.tensor, offset=row.offset,
                   ap=[[0, parts]] + list(row.ap[1:]))


def _patch_act_tables():
    """Strip Exp/Ln from the earlier table sets so both resolve to the shared
    natural_log_exp_and_others set -> no ACT table reload between Ln and Exp
    (set ids keep their positions, only membership changes)."""
    from concourse import hw_specs
    import concourse.bacc as bacc_mod
    if getattr(bacc_mod, "_act_tables_patched", False):
        return
    orig = hw_specs.get_activation_tables

    def patched(arch):
        t = {}
        for k, v in orig(arch).items():
            if k in ("exp_and_others", "natural_log"):
                v = v - {AF.Exp, AF.Ln}
            t[k] = v
        return t

    bacc_mod.get_activation_tables = patched
    bacc_mod._act_tables_patched = True


def build(depth=DEPTH):
    _patch_act_tables()
    nc = bacc.Bacc("TRN2", target_bir_lowering=False, debug=False,
                   enable_asserts=False, num_devices=N_CORES)

    g_d = nc.dram_tensor("g_fm", [DIM, N], FP, kind="ExternalInput").ap()
    x_d = nc.dram_tensor("x_fm", [DIM, TOWN], FP, kind="ExternalInput").ap()
    wdr = {}
    for nm, rows, cols in [("Wq", DIM, INNER), ("Wk", DIM, INNER),
                           ("Wv", DIM, INNER), ("Wo_ca", INNER, DIM),
                           ("Wq_sa", DIM, INNER), ("Wk_sa", DIM, INNER),
                           ("Wv_sa", DIM, INNER), ("Wo_sa", INNER, DIM),
                           ("W1", DIM, MLP), ("W2", MLP, DIM)]:
        wdr[nm] = nc.dram_tensor(nm, [depth, rows, cols], BF,
                                 kind="ExternalInput").ap()
    w1cs_d = nc.dram_tensor("W1cs", [depth, 1, MLP], BF,
                            kind="ExternalInput").ap()
    wcs_d = nc.dram_tensor("Wcs", [depth, 4, INNER], BF,
                           kind="ExternalInput").ap()
    out_d = nc.dram_tensor("x_out", [DIM, TOWN], FP, kind="ExternalOutput").ap()

    with tile.TileContext(nc) as tc:
        with (
            tc.tile_pool(name="persist", bufs=1) as persist,
            tc.tile_pool(name="big", bufs=1) as bigp,
            tc.tile_pool(name="act", bufs=1) as act,
            tc.tile_pool(name="actn", bufs=2) as actn,
            tc.tile_pool(name="wnext", bufs=2) as wnp,
            tc.tile_pool(name="zn", bufs=2) as znp,
            tc.tile_pool(name="oth", bufs=1) as othp,
            tc.tile_pool(name="pt", bufs=8) as ptp,
            tc.tile_pool(name="sq", bufs=1) as sqp,
            tc.tile_pool(name="w", bufs=4) as wp,
            tc.tile_pool(name="wff", bufs=1) as wffp,
            tc.tile_pool(name="srow", bufs=4) as srow,
            tc.tile_pool(name="den", bufs=2) as denp,
            tc.tile_pool(name="nst", bufs=4) as nstp,
            tc.tile_pool(name="brow", bufs=2) as browp,
            tc.tile_pool(name="ps_mm", bufs=2, space="PSUM") as ps_mm,
            tc.tile_pool(name="ps_ss", bufs=4, space="PSUM") as ps_ss,
            tc.tile_pool(name="ps_pv", bufs=2, space="PSUM") as ps_pv,
            tc.tile_pool(name="dram", bufs=2, space="DRAM") as dramp,
        ):
            ones = persist.tile([P, P], FP)   # ones matrix: replicated stats
            nc.vector.memset(ones[:], 1.0)
            ones_bf = persist.tile([P, P], BF)
            nc.vector.memset(ones_bf[:], 1.0)
            eps_col = persist.tile([P, 1], FP)
            nc.vector.memset(eps_col[:], EPS)
            dummy_act = persist.tile([1, 1], FP)
            nc.vector.memset(dummy_act[:], 1.0)
            x_own = persist.tile([P, KC, TOWN], FP)          # x^T own half
            nc.sync.dma_start(x_own[:], _rearr(x_d))
            xb_own = persist.tile([P, KC, TOWN], BF)         # bf16 shadow
            for kc in range(KC):
                nc.scalar.activation(xb_own[:, kc, :], x_own[:, kc, :],
                                     AF.Copy)
            gn = persist.tile([P, KC, N], BF)                # normalize(g)^T

            def hoist_table(func):
                """Tiny dummy activation: forces the ACT table load for
                `func`'s set to happen HERE in the ScalarE queue instead of
                at the next real use (hoists it off the critical chain)."""
                nc.scalar.activation(dummy_act[:], dummy_act[:], func)

            def stats_fm(stat_src, stat_ones):
                """Per-token LN stats of a [P, KC, 512] FM tile.

                The arithmetic chain runs on ScalarE (PSUM evac with scale)
                + GpSimd (idle engine, SBUF-only ops) so stats never queue
                behind attention-tail DVE work.

                Returns (a, c, cb, aT): a = rstd row (replicated, fp32),
                cb = bf16 -mean*rstd row (K=1 matmul operand), aT =
                token-major rstd columns (tiny DRAM transpose bounce)."""
                T = stat_src.shape[2]
                sq = sqp.tile([P, KC, T], stat_src.dtype, tag="sq")
                for kc in range(KC):
                    nc.gpsimd.tensor_tensor(sq[:, kc, :], stat_src[:, kc, :],
                                            stat_src[:, kc, :], OP.mult)
                s_ps = ps_mm.tile([P, T], FP, tag="mm")
                q_ps = ps_mm.tile([P, T], FP, tag="mm")
                for kc in range(KC):
                    nc.tensor.matmul(s_ps[:], stat_ones[:],
                                     stat_src[:, kc, :],
                                     start=kc == 0, stop=kc == KC - 1)
                for kc in range(KC):
                    nc.tensor.matmul(q_ps[:], stat_ones[:], sq[:, kc, :],
                                     start=kc == 0, stop=kc == KC - 1)
                nm = srow.tile([P, T], FP, tag="srow")
                ms = srow.tile([P, T], FP, tag="srow")
                nc.scalar.activation(nm[:], s_ps[:], AF.Copy,
                                     scale=-1.0 / DIM)
                nc.scalar.activation(ms[:], q_ps[:], AF.Copy, scale=1.0 / DIM)
                var = srow.tile([P, T], FP, tag="srow")
                nc.gpsimd.tensor_tensor(var[:], nm[:], nm[:], OP.mult)
                nc.gpsimd.tensor_tensor(var[:], ms[:], var[:], OP.subtract)
                a = nstp.tile([P, T], FP, tag="sta")
                c = nstp.tile([P, T], FP, tag="stc", bufs=2)
                nc.scalar.activation(a[:], var[:], AF.Ln, bias=eps_col[:])
                nc.scalar.activation(a[:], a[:], AF.Exp, scale=-0.5)
                nc.gpsimd.tensor_tensor(c[:], nm[:], a[:], OP.mult)
                cb = nstp.tile([1, T], BF, tag="stcb")
                nc.gpsimd.tensor_copy(out=cb[:], in_=c[0:1, :])
                ar = dramp.tile([1, T], FP, tag="arow")
                nc.sync.dma_start(ar[:], a[0:1, :])
                aT = nstp.tile([P, T // P], FP, tag="staT")
                nc.sync.dma_start(
                    aT[:], ar[0, :].rearrange("(o p) -> p o", p=P))
                return (a, c, cb, aT)

            def norm_apply(src, a, c, dst):
                """dst(bf16) = src*a + c, FM tiles [P, KC, 512] (GpSimd)."""
                tmp = srow.tile([P, 512], FP, tag="srow")
                for kc in range(KC):
                    nc.gpsimd.tensor_tensor(tmp[:], src[:, kc, :],
                                            a[:], OP.mult)
                    nc.gpsimd.tensor_tensor(dst[:, kc, :], tmp[:], c[:],
                                            OP.add)

            def norm_fm_g(src, dst):
                """dst(bf16) = normalize(src fp32) for the static g (T=N)."""
                for c0 in range(0, N, 512):
                    sq = sqp.tile([P, KC, 512], BF, tag="sq")
                    for kc in range(KC):
                        nc.vector.tensor_tensor(sq[:, kc, :],
                                                src[:, kc, c0:c0 + 512],
                                                src[:, kc, c0:c0 + 512],
                                                OP.mult)
                    s_ps = ps_mm.tile([P, 512], FP, tag="mm")
                    q_ps = ps_mm.tile([P, 512], FP, tag="mm")
                    for kc in range(KC):
                        nc.tensor.matmul(s_ps[:], ones[:],
                                         src[:, kc, c0:c0 + 512],
                                         start=kc == 0, stop=kc == KC - 1)
                    for kc in range(KC):
                        nc.tensor.matmul(q_ps[:], ones_bf[:], sq[:, kc, :],
                                         start=kc == 0, stop=kc == KC - 1)
                    nm = srow.tile([P, 512], FP, tag="srow")
                    ms = srow.tile([P, 512], FP, tag="srow")
                    nc.vector.tensor_scalar_mul(nm[:], s_ps[:], -1.0 / DIM)
                    nc.vector.tensor_scalar_mul(ms[:], q_ps[:], 1.0 / DIM)
                    var = srow.tile([P, 512], FP, tag="srow")
                    nc.vector.tensor_tensor(var[:], nm[:], nm[:], OP.mult)
                    nc.vector.tensor_tensor(var[:], ms[:], var[:], OP.subtract)
                    a = srow.tile([P, 512], FP, tag="srow")
                    c = srow.tile([P, 512], FP, tag="srow")
                    nc.scalar.activation(a[:], var[:], AF.Ln, bias=eps_col[:])
                    nc.scalar.activation(a[:], a[:], AF.Exp, scale=-0.5)
                    nc.vector.tensor_tensor(c[:], nm[:], a[:], OP.mult)
                    tmp = srow.tile([P, 512], FP, tag="srow")
                    for kc in range(KC):
                        nc.vector.tensor_tensor(tmp[:],
                                                src[:, kc, c0:c0 + 512],
                                                a[:], OP.mult)
                        nc.vector.tensor_tensor(dst[:, kc, c0:c0 + 512],
                                                tmp[:], c[:], OP.add)

            def load_w(dram_slice):
                t = wp.tile([P, KC, 512], BF, tag="w")
                nc.sync.dma_start(t[:], _rearr(dram_slice))
                return t

            def gemm_fm(w_tile, src, dst, T, t_dst0=0, t_src0=0, evac="act"):
                """dst[:, ft, t_dst0+t] = (W^T @ src), FM output, bf16.

                evac picks the PSUM->SBUF engine: "act" for GEMM/norm phases
                (ScalarE idle there), "dve" near attention (ScalarE is busy
                with softmax exp and would stall the PV pipeline)."""
                Fts = w_tile.shape[2] // P
                for ft in range(Fts):
                    for t0 in range(0, T, 512):
                        ps = ps_mm.tile([P, 512], FP, tag="mm")
                        for kc in range(KC):
                            nc.tensor.matmul(
                                ps[:], w_tile[:, kc, ft * P:(ft + 1) * P],
                                src[:, kc, t_src0 + t0:t_src0 + t0 + 512],
                                start=kc == 0, stop=kc == KC - 1)
                        dsl = dst[:, ft, t_dst0 + t0:t_dst0 + t0 + 512]
                        if evac == "act":
                            nc.scalar.activation(dsl, ps[:], AF.Copy)
                        else:
                            nc.vector.tensor_copy(out=dsl, in_=ps[:])

            def gemm_fm_folded(w_tile, src_bf, dst, wcs, wi, stats,
                               t_dst0=0, evac="dve"):
                """FM GEMM on the RAW bf16 x-shadow; the per-token normalize
                (a, c) is folded in: c via a K=1 matmul against the weight
                column sums, a via the evacuation multiply. Removes the LN
                apply from the GEMM critical path entirely."""
                a, c, cb, aT = stats
                Fts = w_tile.shape[2] // P
                for ft in range(Fts):
                    ps = ps_mm.tile([P, 512], FP, tag="mm")
                    for kc in range(KC):
                        nc.tensor.matmul(ps[:], w_tile[:, kc, ft * P:(ft + 1) * P],
                                         src_bf[:, kc, :],
                                         start=kc == 0, stop=False)
                    nc.tensor.matmul(ps[:], wcs[0:1, wi, ft * P:(ft + 1) * P],
                                     cb[:], start=False, stop=True)
                    dsl = dst[:, ft, t_dst0:t_dst0 + 512]
                    nc.vector.tensor_tensor(dsl, ps[:], a[:], OP.mult)

            def gemm_vcat_folded(w_tile, src_bf, vcat, wcs, wi, stats, tt0=0):
                a, c, cb, aT = stats
                for tt in range(KC):
                    ps = ps_mm.tile([P, 512], FP, tag="mm")
                    for kc in range(KC):
                        nc.tensor.matmul(ps[:], src_bf[:, kc, tt * P:(tt + 1) * P],
                                         w_tile[:, kc, :],
                                         start=kc == 0, stop=False)
                    nc.tensor.matmul(ps[:], cb[0:1, tt * P:(tt + 1) * P],
                                     wcs[0:1, wi, :], start=False, stop=True)
                    nc.vector.tensor_scalar_mul(
                        vcat[:, tt0 + tt, :, 0:DHEAD],
                        ps.rearrange("p (h d) -> p h d", h=HEADS),
                        aT[:, tt:tt + 1])

            def attention(qT, kT, vcat, merged, mid_cb=None, mid_kc=KC):
                """merged (FM bf16 [128, KC, 512]) = softmax(qk^T*scale)@v.

                mid_cb is invoked after `mid_kc` own-half k-chunks of the
                first head pair: the emitted instructions (other-half k/v
                GEMMs, which wait on the AllReduce) land behind own-half PE
                work in the static per-engine order, hiding the exchange."""
                NP = NT // 2
                for hp in range(HP):
                    pv0 = ps_pv.tile([DHEAD + 2, 512], FP, tag="pv")
                    pv1 = ps_pv.tile([DHEAD + 2, 512], FP, tag="pv")
                    pairs = [None] * NP

                    def emit_pv(p, pv0=pv0, pv1=pv1, pairs=pairs, vcat=vcat):
                        # fp8 DoubleRow: two key-chunks (k-tiles) per matmul
                        nc.tensor.matmul(pv0[:],
                                         vcat[:, 2 * p:2 * p + 2, 2 * hp, :],
                                         pairs[p][0][:], perf_mode=DR,
                                         start=p == 0, stop=p == NP - 1)
                        nc.tensor.matmul(pv1[:],
                                         vcat[:, 2 * p:2 * p + 2,
                                              2 * hp + 1, :],
                                         pairs[p][1][:], perf_mode=DR,
                                         start=p == 0, stop=p == NP - 1)

                    for kc in range(NT):
                        if mid_cb is not None and hp == 0 and kc == mid_kc:
                            mid_cb()
                            mid_cb = None
                        ss0 = ps_ss.tile([P, 512], FP, tag="ss")
                        ss1 = ps_ss.tile([P, 512], FP, tag="ss")
                        nc.tensor.matmul(ss0[:],
                                         kT[0:DHEAD, hp, kc * P:(kc + 1) * P],
                                         qT[0:DHEAD, hp, :],
                                         start=True, stop=True)
                        nc.tensor.matmul(ss1[:],
                                         kT[DHEAD:P, hp, kc * P:(kc + 1) * P],
                                         qT[DHEAD:P, hp, :],
                                         start=True, stop=True)
                        if kc % 2 == 0:
                            ptA = ptp.tile([P, 2, 512], F8, tag="pt",
                                           name="ptA")
                            ptB = ptp.tile([P, 2, 512], F8, tag="pt",
                                           name="ptB")
                            pairs[kc // 2] = (ptA, ptB)
                        ptA, ptB = pairs[kc // 2]
                        nc.scalar.activation(ptA[:, kc % 2, :], ss0[:],
                                             AF.Exp, scale=SCALE)
                        nc.scalar.activation(ptB[:, kc % 2, :], ss1[:],
                                             AF.Exp, scale=SCALE)
                        # lag PV one pair behind so it never head-of-line
                        # blocks on its own exp
                        if kc >= 3 and kc % 2 == 1:
                            emit_pv((kc - 3) // 2)
                    emit_pv(NP - 2)
                    emit_pv(NP - 1)
                    # evacuate PV unnormalized immediately (releases the pv
                    # psum slots for the next pair); normalize in place after
                    den_sb = denp.tile([1, 1024], FP, tag="densb")
                    nc.vector.tensor_copy(out=den_sb[:, 0:512],
                                          in_=pv0[DHEAD:DHEAD + 1, :])
                    nc.vector.tensor_copy(out=den_sb[:, 512:1024],
                                          in_=pv1[DHEAD:DHEAD + 1, :])
                    nc.vector.tensor_copy(out=merged[0:DHEAD, hp, :],
                                          in_=pv0[0:DHEAD, :])
                    nc.vector.tensor_copy(out=merged[DHEAD:P, hp, :],
                                          in_=pv1[0:DHEAD, :])
                    r01 = denp.tile([1, 1024], FP, tag="den")
                    nc.vector.reciprocal_approx_fast(out=r01[:], in_=den_sb[:])
                    rd = dramp.tile([2, 512], FP, tag="rrow")
                    nc.sync.dma_start(rd[:].rearrange("a b -> (a b)")[None, :],
                                      r01[:])
                    rb = browp.tile([P, 512], FP, tag="brow")
                    nc.sync.dma_start(rb[0:DHEAD, :], _bcast(rd[0:1, :], DHEAD))
                    nc.sync.dma_start(rb[DHEAD:P, :], _bcast(rd[1:2, :], DHEAD))
                    nc.vector.tensor_tensor(merged[0:DHEAD, hp, :],
                                            merged[0:DHEAD, hp, :],
                                            rb[0:DHEAD, :], OP.mult)
                    nc.vector.tensor_tensor(merged[DHEAD:P, hp, :],
                                            merged[DHEAD:P, hp, :],
                                            rb[DHEAD:P, :], OP.mult)

            def gemm_residual(w_tile, src):
                """x_own += src^T @ W  (W [DIM, DIM] natural as lhsT)."""
                for d in range(KC):
                    ps = ps_mm.tile([P, 512], FP, tag="mm")
                    for kc in range(KC):
                        nc.tensor.matmul(ps[:], w_tile[:, kc, d * P:(d + 1) * P],
                                         src[:, kc, :],
                                         start=kc == 0, stop=kc == KC - 1)
                    nc.vector.tensor_tensor(x_own[:, d, :], ps[:],
                                            x_own[:, d, :], OP.add)
                    nc.scalar.activation(xb_own[:, d, :], x_own[:, d, :],
                                         AF.Copy)

            def ff(w1, w2, zf):
                h = bigp.tile([P, MC, TOWN], BF, tag="h")
                # W2 accumulates all 4 output d-tiles in parallel (borrowing
                # the attention ss psum slots, idle during FF) with the
                # k-chunk loop OUTERMOST: each W2 matmul issues as soon as
                # its gelu chunk lands instead of after the whole h tensor.
                accs = [ps_ss.tile([P, 512], FP, tag="ss", name=f"acc{d}")
                        for d in range(KC)]
                for ft in range(MC):
                    ps = ps_mm.tile([P, 512], FP, tag="mm")
                    for kc in range(KC):
                        nc.tensor.matmul(ps[:], w1[:, kc, ft * P:(ft + 1) * P],
                                         zf[:, kc, :],
                                         start=kc == 0, stop=kc == KC - 1)
                    nc.scalar.activation(h[:, ft, :], ps[:], AF.Gelu)
                    for d in range(KC):
                        nc.tensor.matmul(accs[d][:],
                                         w2[:, ft, d * P:(d + 1) * P],
                                         h[:, ft, :],
                                         start=ft == 0, stop=ft == MC - 1)
                # hoist the ln/exp table reload behind the W2 tail: the next
                # phase's first Ln would otherwise pay it on the stats chain
                hoist_table(AF.Ln)
                for d in range(KC):
                    nc.vector.tensor_tensor(x_own[:, d, :], accs[d][:],
                                            x_own[:, d, :], OP.add)
                    nc.scalar.activation(xb_own[:, d, :], x_own[:, d, :],
                                         AF.Copy)

            def exchange():
                """Pairwise AllGather of the raw bf16 x shadow."""
                bi = dramp.tile([DIM, TOWN], BF, tag="agin")
                bo = dramp.tile([2 * DIM, TOWN], BF, tag="agout")
                nc.sync.dma_start(_rearr(bi[:]), xb_own[:])
                nc.gpsimd.collective_compute(
                    "AllGather", OP.bypass, ins=[bi.opt()], outs=[bo.opt()],
                    replica_groups=RG)
                return bo

            def assemble_other(bo):
                """other = block0 + block1 - own  (position-independent,
                on the otherwise-idle GpSimd so the attention-tail DVE
                queue cannot delay it)."""
                zo = othp.tile([P, KC, TOWN], BF, tag="znoth")
                bb = othp.tile([P, KC, TOWN], BF, tag="bblk")
                nc.sync.dma_start(zo[:], _rearr(bo[0:DIM, :]))
                nc.sync.dma_start(bb[:], _rearr(bo[DIM:2 * DIM, :]))
                for kc in range(KC):
                    nc.gpsimd.tensor_tensor(zo[:, kc, :], zo[:, kc, :],
                                            bb[:, kc, :], OP.add)
                    nc.gpsimd.tensor_tensor(zo[:, kc, :], zo[:, kc, :],
                                            xb_own[:, kc, :], OP.subtract)
                return zo

            def ham_warm(n=16):
                """Dummy bf16 matmuls to keep the PE HAM clock-gate hot
                across norm gaps where no real PE work is available."""
                ps = ps_ss.tile([P, 512], FP, tag="ss")
                for _ in range(n):
                    nc.tensor.matmul(ps[:], ones_bf[:], gn[:, 0, 0:512],
                                     start=True, stop=True)

            # ---- prologue: first exchange + static gn = normalize(g) ----
            g_fm = bigp.tile([P, KC, N], FP, tag="h")  # reuse h slot
            nc.sync.dma_start(g_fm[:], _rearr(g_d))
            hoist_table(AF.Ln)
            bo_ca = exchange()
            st_end = stats_fm(xb_own, ones_bf)
            norm_fm_g(g_fm, gn)

            for i in range(depth):
                # ======== relational cross attention ========
                if i == 0:
                    wq = load_w(wdr["Wq"][0])
                    wk = load_w(wdr["Wk"][0])
                    qT = actn.tile([P, KC, 512], BF, tag="qT")
                    kT = actn.tile([P, KC, N], BF, tag="kT")
                    gemm_fm(wq, gn, qT, 512)          # own queries (local)
                    gemm_fm(wk, gn, kT, N)            # all keys (g static)
                else:
                    qT, kT = qT_next, kT_next
                # all of this layer's weight DMAs up front, in consumption
                # order: they roll through the pools as prefetch so no GEMM
                # waits on a just-issued transfer.
                wv = load_w(wdr["Wv"][i])
                if i + 1 < depth:  # next-layer fill weights
                    wk_n = wnp.tile([P, KC, 512], BF, tag="wn")
                    nc.sync.dma_start(wk_n[:], _rearr(wdr["Wk"][i + 1]))
                    wq_n = wnp.tile([P, KC, 512], BF, tag="wn")
                    nc.sync.dma_start(wq_n[:], _rearr(wdr["Wq"][i + 1]))
                woc = load_w(wdr["Wo_ca"][i])
                w1 = wffp.tile([P, KC, MLP], BF, tag="w1")
                nc.sync.dma_start(w1[:], _rearr(wdr["W1"][i]))
                w2 = wffp.tile([P, MC, DIM], BF, tag="w2")
                nc.sync.dma_start(w2[:], _rearr(wdr["W2"][i]))
                w1cs = othp.tile([1, MLP], BF, tag="w1cs")
                nc.sync.dma_start(w1cs[:], w1cs_d[i])
                wcs = othp.tile([1, 4, INNER], BF, tag="wcs")
                nc.sync.dma_start(wcs[:], wcs_d[i])
                wqs = load_w(wdr["Wq_sa"][i])
                wks = load_w(wdr["Wk_sa"][i])
                wvs = load_w(wdr["Wv_sa"][i])
                wos = load_w(wdr["Wo_sa"][i])
                vcat = act.tile([P, NT, HEADS, DHEAD + 2], F8, tag="vcat")
                nc.vector.memset(vcat[:, :, :, DHEAD:DHEAD + 1], 1.0)
                nc.vector.memset(vcat[:, :, :, DHEAD + 1:DHEAD + 2], 0.0)
                gemm_vcat_folded(wv, xb_own, vcat, wcs, 0, st_end)
                merged = act.tile([P, KC, 512], BF, tag="merged")

                def ca_mid(bo=bo_ca, wv=wv, wcs=wcs, vc=vcat):
                    zo = assemble_other(bo)
                    st_o = stats_fm(zo, ones_bf)
                    gemm_vcat_folded(wv, zo, vc, wcs, 0, st_o, tt0=KC)
                attention(qT, kT, vcat, merged, mid_cb=ca_mid, mid_kc=KC)
                gemm_residual(woc, merged)
                # ======== feed-forward 1 ========
                # fill the xb-cast + stats gap with next layer's first k half
                ham_warm(4)
                if i + 1 < depth:
                    kT_next = actn.tile([P, KC, N], BF, tag="kT")
                    gemm_fm(wk_n, gn, kT_next, 512)
                st_f1 = stats_fm(xb_own, ones_bf)
                zf = znp.tile([P, KC, TOWN], BF, tag="znown")
                norm_apply(x_own, st_f1[0], st_f1[1], zf)
                hoist_table(AF.Gelu)
                ham_warm(8)
                ff(w1, w2, zf)
                # ======== self attention ========
                bo_sa = exchange()
                ham_warm(4)
                # fill: next layer's queries (gn is static)
                if i + 1 < depth:
                    qT_next = actn.tile([P, KC, 512], BF, tag="qT")
                    gemm_fm(wq_n, gn, qT_next, 512)
                st1 = stats_fm(xb_own, ones_bf)
                ham_warm(6)
                qTs = actn.tile([P, KC, 512], BF, tag="qT")
                kTs = actn.tile([P, KC, N], BF, tag="kT")
                vcats = act.tile([P, NT, HEADS, DHEAD + 2], F8, tag="vcat")
                nc.vector.memset(vcats[:, :, :, DHEAD:DHEAD + 1], 1.0)
                nc.vector.memset(vcats[:, :, :, DHEAD + 1:DHEAD + 2], 0.0)
                gemm_fm_folded(wqs, xb_own, qTs, wcs, 1, st1)
                gemm_fm_folded(wks, xb_own, kTs, wcs, 2, st1)
                gemm_vcat_folded(wvs, xb_own, vcats, wcs, 3, st1)
                mergeds = act.tile([P, KC, 512], BF, tag="merged")

                def sa_mid(bo=bo_sa, wk_=wks, wv_=wvs, wcs=wcs, kt=kTs,
                           vc=vcats):
                    zo1 = assemble_other(bo)
                    st_o = stats_fm(zo1, ones_bf)
                    gemm_fm_folded(wk_, zo1, kt, wcs, 2, st_o, t_dst0=512)
                    gemm_vcat_folded(wv_, zo1, vc, wcs, 3, st_o, tt0=KC)
                attention(qTs, kTs, vcats, mergeds, mid_cb=sa_mid, mid_kc=KC)
                gemm_residual(wos, mergeds)
                # ======== feed-forward 2 ========
                # fill: next layer's second k half
                ham_warm(4)
                if i + 1 < depth:
                    gemm_fm(wk_n, gn, kT_next, 512, t_dst0=512, t_src0=512)
                st_f2 = stats_fm(xb_own, ones_bf)
                zf2 = znp.tile([P, KC, TOWN], BF, tag="znown")
                norm_apply(x_own, st_f2[0], st_f2[1], zf2)
                hoist_table(AF.Gelu)
                ham_warm(8)
                ff(w1, w2, zf2)
                if i + 1 < depth:
                    bo_ca = exchange()
                    ham_warm(12)
                    st_end = stats_fm(xb_own, ones_bf)
                    ham_warm(8)

            nc.sync.dma_start(_rearr(out_d[:]), x_own[:])

    nc.compile()
    return nc


# ======================= host side =======================

_NC_CACHE = {}


def _get_nc(depth=DEPTH):
    if depth not in _NC_CACHE:
        _NC_CACHE[depth] = build(depth)
    return _NC_CACHE[depth]


def _prep_inputs(inputs, depth=DEPTH):
    import ml_dtypes
    bf16 = ml_dtypes.bfloat16
    f32 = lambda a: np.asarray(a, np.float32)
    g, x = f32(inputs["g"]), f32(inputs["x"])
    lng_s, lnx_s = f32(inputs["lng_s"]), f32(inputs["lnx_s"])
    ln1_s, lnf_s = f32(inputs["ln1_s"]), f32(inputs["lnf_s"])
    # all additive biases must be zero for this kernel (they are, per
    # setup_inputs); LN scales are folded into the adjacent weights.
    for k in ("lng_b", "lnx_b", "ln1_b", "lnf_b", "bv",
              "bo_ca", "bo_sa", "b1", "b2"):
        assert np.abs(f32(inputs[k])).max() == 0.0, f"nonzero bias {k}"
    Wq = lng_s[:, :, None] * f32(inputs["Wq"])
    Wk = lng_s[:, :, None] * f32(inputs["Wk"])
    Wv = lnx_s[:, :, None] * f32(inputs["Wv"])
    Wqkv = ln1_s[:, :, None] * f32(inputs["Wqkv"])
    W1 = lnf_s[:, :, None] * f32(inputs["W1"])
    c = lambda a: np.ascontiguousarray(a.astype(bf16))
    weights = {
        "Wq": c(Wq[:depth]), "Wk": c(Wk[:depth]), "Wv": c(Wv[:depth]),
        "Wo_ca": c(f32(inputs["Wo_ca"])[:depth]),
        "Wq_sa": c(Wqkv[:depth, :, 0:INNER]),
        "Wk_sa": c(Wqkv[:depth, :, INNER:2 * INNER]),
        "Wv_sa": c(Wqkv[:depth, :, 2 * INNER:3 * INNER]),
        "Wo_sa": c(f32(inputs["Wo_sa"])[:depth]),
        "W1": c(W1[:depth]), "W2": c(f32(inputs["W2"])[:depth]),
        "W1cs": c(W1[:depth].astype(bf16).astype(np.float32)
                  .sum(axis=1, keepdims=True)),
    }
    wcs = np.stack([
        weights["Wv"].astype(np.float32).sum(axis=1),
        weights["Wq_sa"].astype(np.float32).sum(axis=1),
        weights["Wk_sa"].astype(np.float32).sum(axis=1),
        weights["Wv_sa"].astype(np.float32).sum(axis=1),
    ], axis=1)
    weights["Wcs"] = c(wcs)
    in_maps = []
    cc = np.ascontiguousarray
    for core in range(N_CORES):
        b, h = core // 2, core % 2
        own = slice(h * TOWN, (h + 1) * TOWN)
        oth = slice((1 - h) * TOWN, (2 - h) * TOWN)
        g_local = np.concatenate([g[b, own], g[b, oth]], axis=0)  # local order
        m = dict(weights)
        m["g_fm"] = cc(g_local.T)
        m["x_fm"] = cc(x[b, own].T)
        in_maps.append(m)
    return in_maps


def _assemble(results):
    out = np.empty((B, N, DIM), np.float32)
    for core in range(N_CORES):
        b, h = core // 2, core % 2
        out[b, h * TOWN:(h + 1) * TOWN] = results[core]["x_out"].T
    return out


def run(inputs, depth=DEPTH, trace=False, tmpdir=None):
    nc = _get_nc(depth)
    in_maps = _prep_inputs(inputs, depth)
    res = bass_utils.run_bass_kernel_spmd(
        nc, in_maps, core_ids=list(range(N_CORES)), trace=trace, tmpdir=tmpdir)
    return _assemble(res.results), res


def kernel(**inputs) -> np.ndarray:
    out, _ = run(inputs)
    return out
